# revision 40
# baseline (speedup 1.0000x reference)
"""Dilated self-attention TRN2 kernel (nn_DilatedSelfAttention).

Problem (hardcoded — self-contained):
  x (4, 8192, 128) f32; Wq/Wk/Wv (128,128) f32; indices (14336) i64.
  WS=[2048,4096,8192], RS=[1,2,4], HEAD_IDX=1 -> 7 segments of 2048 per batch:
    seg0..3: windows [2048t, 2048(t+1))           (stride 1)
    seg4:    1 + 2*i, i<2048   (odd of [0,4096))  (stride 2)
    seg5:    4097 + 2*i        (odd of [4096,8192))
    seg6:    1 + 4*i           (p%4==1)           (stride 4)
  Each segment: causal softmax attention (per-segment row max subtracted),
  outputs mixed position-wise weighted by softmax denominators:
    out[p] = sum_seg (expS @ v)[p] / sum_seg denom[p]   (per-seg max shifts
    folded into both numerator and denominator — matches reference exactly).

Sharding: core pair (2b, 2b+1) owns batch b. Each segment is split into two
half-pieces by query 128-tile parity (delta=0: even qtiles, delta=1: odd).
Every core runs SEVEN structurally identical pieces (uniform SPMD program);
the only per-core data differences are the gathered inputs, the diag masks,
and a dynamic column offset (128*delta) for the output scatter.

All PE inputs are f16 (prepared on host: x^T per segment packed with the
additive -3e4 diag mask into one [C, 2304] DMA payload, M = WqWk^T/sqrt(C),
Wv; -3e4 is equivalent to -1e9 since exp underflows to 0 either way).
Per piece (segment context S=2048, local queries QL=1024 in 8 slots of 128):
  q' = x_seg @ M   [f16 matmul, PSUM->f16 qpt via DVE+ACT]
  v  = x_seg @ Wv  [f16, PSUM->f16 vsl via DVE+ACT]
  slot j: S-row = q'_j @ x over 256(j+1) keys, computed as 512-col blocks
    into SINGLE-BANK [128,512] PSUM tiles (6-buffer ring; one bank each so
    ~6 blocks stay in flight — the release->matmul->rowmax->exp PSUM ring is
    the throughput limiter otherwise), additive diag/pad mask via
    ident16@mask matmul into the last block,
    per-block rowmax partial (DVE) -> negmx -> per-block exp with bias=-mx
    and fused denominator accum (ACT) -> E f16,
    one blocked DMA-xbar transpose E -> ET[k-chunk, q] per slot,
    EV: out^T[c, 128 q] accumulated over the 2(j+1) causal k-chunks (the
    extra chunk vs delta=0's true need holds exp(masked)=0, keeping the
    program uniform across cores).
  EV matmuls are drained with an 8-slot lag AND carry a nosync ordering edge
  onto the most recent slot's S-matmul: the Tile scheduler's internal cost
  model under-estimates DMA-transpose completion (HWDGE 625ns serial issue +
  transfer + ~900ns sem), and without the edge it interleaves EVs early where
  they head-of-line-block the in-order PE queue for ~3us per piece.
  Scatter-add: EV PSUM -> SBUF (DVE), then Pool adds into batch-position
  accumulators at dynamic strided offsets (beta = 128*delta); denominators
  go through a PE transpose + DMA row-flatten (GPSIMD cannot access PSUM).
Pair ReduceScatter sums the two cores' accumulators; each core normalizes and
writes half the batch rows.

Measured (loop-slope on HW, 8 cores): ~191us/iter compute vs ~306us baseline;
TimelineSim (cost-model) 162us with ACT the busiest engine at 74%.
"""
import math
import os
import sys

sys.path.insert(0, "/opt/trn_rl_repo")

import numpy as np

import concourse.bass as bass
import concourse.bacc as bacc
import concourse.mybir as mybir
import concourse.tile as tile
from concourse.bass_utils import run_bass_kernel_spmd
from concourse.masks import make_identity

f32 = mybir.dt.float32
f32r = mybir.dt.float32r
f16 = mybir.dt.float16
i32 = mybir.dt.int32

B, N, C = 4, 8192, 128
S = 2048          # segment length
NCH = 16          # 128-chunks per segment
NSLOT = 8         # q-slots per piece
QL = NSLOT * 128  # 1024 local queries per piece
NPIECE = 7
NEG = -30000.0    # additive mask; exp(s-m+NEG) underflows to 0 exactly

# per piece-slot-index: segment id == piece id; (base, stride) of position map
SEG_BASE = [0, 2048, 4096, 6144, 1, 4097, 1]
SEG_STRIDE = [1, 1, 1, 1, 2, 2, 4]

EV_LAG = 8        # slots between a slot's softmax and its EV matmuls
SW = S + 256      # bx payload cols: x_seg^T (2048) ++ mask (256)


def build_nc(loop_k=None, skip_rs=False, skip=(), unroll_k=None):
    nc = bacc.Bacc(None, target_bir_lowering=False)

    bx7 = nc.dram_tensor("bx7", [NPIECE, C, SW], f16, kind="ExternalInput")
    beta7 = nc.dram_tensor("beta7", [1, NPIECE], i32, kind="ExternalInput")
    Mt = nc.dram_tensor("Mt", [C, C], f16, kind="ExternalInput")
    Wvt = nc.dram_tensor("Wvt", [C, C], f16, kind="ExternalInput")
    out_half = nc.dram_tensor("out_half", [N // 2, C], f32, kind="ExternalOutput")

    HALF = N // 2                      # 4096 positions per core after RS
    NUMSZ = C * HALF                   # 524288
    EXSZ = NUMSZ + HALF                # + DenT half

    with tile.TileContext(nc) as tc:
        with (
            tc.tile_pool(name="fix", bufs=1) as fix,
            tc.tile_pool(name="bx", bufs=2) as bxp,
            tc.tile_pool(name="dr1", bufs=3) as dr1p,
            tc.tile_pool(name="qpr", bufs=2) as qprp,
            tc.tile_pool(name="vsl", bufs=2) as vslp,
            tc.tile_pool(name="msk", bufs=2) as mskp,
            tc.tile_pool(name="ET", bufs=2) as ETp,
            tc.tile_pool(name="E", bufs=6) as Ep,
            tc.tile_pool(name="small", bufs=6) as smp,
            tc.tile_pool(name="dsb", bufs=2) as dsbp,
            tc.tile_pool(name="spool", bufs=7, space="PSUM") as spool,
            tc.tile_pool(name="evp", bufs=1, space="PSUM") as evp,
            tc.tile_pool(name="dram", bufs=1, space="DRAM") as drp,
            tc.tile_pool(name="epi", bufs=1) as epi,
        ):
            # ---- fixed tensors ----
            ident = fix.tile([128, 128], f32)
            make_identity(nc, ident[:])
            ident16 = fix.tile([128, 128], f16)
            nc.gpsimd.tensor_copy(ident16[:], ident[:])

            m16 = fix.tile([C, C], f16)
            wv16 = fix.tile([C, C], f16)
            nc.sync.dma_start(m16[:], Mt[:])
            nc.sync.dma_start(wv16[:], Wvt[:])

            beta_sb = fix.tile([1, NPIECE], i32)
            nc.sync.dma_start(beta_sb[:], beta7[:])

            NumT = fix.tile([C, N], f32)
            DenT = fix.tile([1, N], f32)
            nc.gpsimd.memset(NumT[:], 0.0)
            nc.gpsimd.memset(DenT[:], 0.0)

            exch_in = drp.tile([2, EXSZ], f32)
            exch_out = drp.tile([1, EXSZ], f32)

            def _one_iter(su):
                # ---- per-piece pipeline state ----
                st_bx = [None] * NPIECE
                st_msk = [None] * NPIECE
                st_beta = [None] * NPIECE
                st_qpt = [None] * NPIECE
                st_vsl = [None] * NPIECE
                st_dsl = [None] * NPIECE
                st_ET = [None] * NPIECE
                st_ev = [None] * NPIECE    # (ev_lo, ev_hi) PSUM tiles
                st_numv = [None] * NPIECE
                st_denv = [None] * NPIECE
                st_qxs = [None] * NPIECE
                st_nms = [None] * NPIECE
                st_wrep = [None] * NPIECE
                st_drow = [None] * NPIECE
                anchor = [None]   # last S-matmul inst of most recent slot

                def emit_load(p):
                    bxm = bxp.tile([C, SW], f16, name=f"bx{p}{su}", tag="bx")
                    nc.sync.dma_start(bxm[:], bx7[p])
                    st_bx[p], st_msk[p] = bxm, bxm[:, S:SW]

                    regs = nc.alloc_registers(
                        f"beta_{p}{su}",
                        engines=[mybir.EngineType.Pool, mybir.EngineType.DVE],
                    )
                    nc.regs_load(regs, beta_sb[0:1, p : p + 1])
                    beta = nc.snap(regs, donate=True, min_val=0, max_val=128)
                    st_beta[p] = beta

                    # gather local queries (strided by qtile parity) -> qx;
                    # emitted at load time so it queues on Pool ahead of the
                    # previous pieces' scatter-adds (avoids head-of-line wait)
                    qsrc = (
                        bxm[:, bass.ds(beta, 1920)]
                        .rearrange("p (j i) -> p j i", i=128)[:, 0::2, :]
                    )
                    qx = qprp.tile([C, NSLOT, 128], f16, name=f"qx{p}{su}", tag="qx")
                    nc.gpsimd.tensor_copy(qx[:], qsrc)
                    st_qxs[p] = qx

                def emit_prep(p):
                    bx = st_bx[p]
                    qx = st_qxs[p]
                    beta = st_beta[p]

                    # q' = M @ qx  (PSUM f32 -> f16 qpt via ACT)
                    qpt = qprp.tile([C, QL], f16, tag="qpt", name=f"qpt{p}{su}")
                    for h in range(2):
                        qp_ps = spool.tile([128, 512], f32, tag="s", name=f"qps{p}{h}{su}")
                        nc.tensor.matmul(
                            qp_ps[:],
                            m16[:],
                            qx[:].rearrange("p j i -> p (j i)")[
                                :, 512 * h : 512 * h + 512
                            ],
                            start=True, stop=True, skip_group_check=True,
                        )
                        if h == 0:
                            nc.vector.tensor_copy(qpt[:, 0:512], qp_ps[:])
                        else:
                            nc.scalar.copy(qpt[:, 512:1024], qp_ps[:])

                    # v = x_seg @ Wv  (PSUM f32 -> f16 vsl via Pool)
                    vsl = vslp.tile([128, NCH * 128], f16, name=f"vsl{p}{su}", tag="vsl")
                    for qtr in range(4):
                        v_ps = spool.tile([128, 512], f32, tag="s", name=f"vps{p}{qtr}{su}")
                        for q in range(4):
                            cch = 4 * qtr + q
                            nc.tensor.matmul(
                                v_ps[:, 128 * q : 128 * q + 128],
                                bx[:, 128 * cch : 128 * cch + 128],
                                wv16[:],
                                start=True, stop=True, skip_group_check=True,
                            )
                        if qtr < 3:
                            nc.vector.tensor_copy(
                                vsl[:, 512 * qtr : 512 * qtr + 512], v_ps[:])
                        else:
                            nc.scalar.copy(
                                vsl[:, 512 * qtr : 512 * qtr + 512], v_ps[:])

                    ET = ETp.tile([128, NCH, QL], f16, name=f"ET{p}{su}", tag="ET")
                    denslab = smp.tile([128, NSLOT], f32, tag="denslab", name=f"dsl{p}{su}")
                    st_qpt[p], st_vsl[p], st_dsl[p], st_ET[p] = qpt, vsl, denslab, ET

                    beta = st_beta[p]
                    sstr, sbase = SEG_STRIDE[p], SEG_BASE[p]
                    st_numv[p] = (
                        NumT[:, sbase :: sstr][:, bass.ds(beta, 1920)]
                        .rearrange("p (j i) -> p j i", i=128)[:, 0::2, :]
                    )
                    st_denv[p] = (
                        DenT[:, sbase :: sstr][:, bass.ds(beta, 1920)]
                        .rearrange("p (j i) -> p j i", i=128)[:, 0::2, :]
                    )

                def emit_slot(p, j):
                    bx, mk = st_bx[p], st_msk[p]
                    qpt, denslab, ET = st_qpt[p], st_dsl[p], st_ET[p]
                    ext = 256 * (j + 1)
                    nb = (ext + 511) // 512
                    maxp = smp.tile([128, 4], f32, tag="maxp", name=f"mx{p}{j}{su}")
                    sblocks = []
                    # one single-bank PSUM tile per 512-col block: six blocks
                    # in flight keep the release->matmul->max->exp ring off
                    # the critical path
                    for b in range(nb):
                        w = min(512, ext - 512 * b)
                        stt = spool.tile([128, 512], f32, tag="s", name=f"st{p}_{j}_{b}{su}")
                        sblocks.append((stt, w))
                        mi = nc.tensor.matmul(
                            stt[:, 0:w],
                            qpt[:, 128 * j : 128 * j + 128],
                            bx[:, 512 * b : 512 * b + w],
                            start=True, stop=not (b == nb - 1),
                            skip_group_check=True,
                        )
                        if b == nb - 1:
                            mi = nc.tensor.matmul(
                                stt[:, w - 256 : w],
                                ident16[:],
                                mk,
                                start=False, stop=True, skip_group_check=True,
                            )
                            anchor[0] = mi.ins
                        nc.vector.tensor_reduce(
                            maxp[:, b : b + 1], stt[:, 0:w],
                            axis=mybir.AxisListType.X, op=mybir.AluOpType.max,
                        )
                    negmx = smp.tile([128, 1], f32, tag="negmx", name=f"nm{p}{j}{su}")
                    nc.vector.tensor_reduce(
                        negmx[:], maxp[:, 0:nb],
                        axis=mybir.AxisListType.X, op=mybir.AluOpType.max,
                        negate=True,
                    )

                    Et = Ep.tile([128, S], f16, tag="Et", name=f"Et{p}{j}{su}")
                    denp = smp.tile([128, 4], f32, tag="denp", name=f"dp{p}{j}{su}")
                    for b, (stt, w) in enumerate(sblocks):
                        nc.scalar.activation(
                            Et[:, 512 * b : 512 * b + w],
                            stt[:, 0:w],
                            mybir.ActivationFunctionType.Exp,
                            bias=negmx[:, 0:1], scale=1.0,
                            accum_out=denp[:, b : b + 1],
                        )
                    nc.vector.tensor_reduce(
                        denslab[:, j : j + 1], denp[:, 0:nb],
                        axis=mybir.AxisListType.X, op=mybir.AluOpType.add,
                    )

                    nc.sync.dma_start_transpose(
                        ET[:, 0 : 2 * (j + 1), 128 * j : 128 * j + 128],
                        Et[:, 0:ext],
                    )

                def emit_ev(p, j):
                    vsl, ET = st_vsl[p], st_ET[p]
                    if j % 4 == 0:
                        ev = evp.tile([128, 512], f32, tag="ev",
                                      name=f"ev{p}_{j // 4}{su}")
                        if j < 4:
                            st_ev[p] = [ev, None]
                        else:
                            st_ev[p][1] = ev
                    ev = st_ev[p][j // 4]
                    col = 128 * (j % 4)
                    nch_j = 2 * (j + 1)
                    for cch in range(nch_j):
                        mi = nc.tensor.matmul(
                            ev[:, col : col + 128],
                            vsl[:, 128 * cch : 128 * cch + 128],
                            ET[:, cch, 128 * j : 128 * j + 128],
                            start=(cch == 0),
                            stop=(cch == nch_j - 1),
                            skip_group_check=True,
                        )
                        if cch == 0 and anchor[0] is not None:
                            import bass_rust as _br
                            _br.add_dep_helper(
                                mi.ins, anchor[0], sync=False,
                                reason="keep EV behind the S-matmul stream",
                            )

                def emit_scatter(p, g):
                    # EV PSUM group g (slots 4g..4g+3) -> SBUF, then Pool adds
                    # into the NumT view (GPSIMD cannot touch PSUM)
                    ev = st_ev[p][g]
                    evs = dr1p.tile([C, 512], f32, tag="evs", name=f"evs{p}{g}{su}")
                    nc.vector.tensor_copy(evs[:], ev[:])
                    numv = st_numv[p][:, 4 * g : 4 * g + 4, :]
                    nc.gpsimd.tensor_tensor(
                        numv, numv,
                        evs[:].rearrange("p (j i) -> p j i", i=128),
                        op=mybir.AluOpType.add,
                    )

                def emit_wden(p):
                    denslab = st_dsl[p]
                    dslT = evp.tile([NSLOT, 128], f32, tag="ev", name=f"dslT{p}{su}")
                    nc.tensor.transpose(dslT[:], denslab[:, 0:NSLOT], ident[:])
                    dsl_sb = dsbp.tile([NSLOT, 128], f32, tag="dslsb", name=f"dsb{p}{su}")
                    nc.vector.tensor_copy(dsl_sb[:], dslT[:])
                    denrow = dr1p.tile([1, QL], f32, tag="denrow", name=f"drow{p}{su}")
                    nc.sync.dma_start(denrow[:], dsl_sb[:])
                    st_drow[p] = denrow

                def emit_denadd(p):
                    denv = st_denv[p]
                    nc.gpsimd.tensor_tensor(
                        denv, denv,
                        st_drow[p][:].rearrange("p (j i) -> p j i", i=128),
                        op=mybir.AluOpType.add,
                    )

                # ---- main software-pipelined emission ----
                SLOT_ORDER = list(range(NSLOT))
                pend = []
                ev_done = {}

                def drain_one():
                    pp, jj = pend.pop(0)
                    emit_ev(pp, jj)
                    g = jj // 4
                    k = (pp, g)
                    ev_done[k] = ev_done.get(k, 0) + 1
                    if ev_done[k] == 4:
                        emit_scatter(pp, g)
                        if ev_done.get((pp, 0), 0) == 4 and ev_done.get((pp, 1), 0) == 4:
                            emit_denadd(pp)

                emit_load(0)
                emit_prep(0)
                for p in range(NPIECE):
                    if p + 1 < NPIECE:
                        emit_load(p + 1)
                    for j in SLOT_ORDER:
                        emit_slot(p, j)
                        pend.append((p, j))
                        while len(pend) > EV_LAG:
                            drain_one()
                    emit_wden(p)
                    if p + 1 < NPIECE:
                        emit_prep(p + 1)
                while pend:
                    drain_one()

                # ---- exchange: ReduceScatter over the pair ----
                for h in range(2 if not skip_rs else 0):
                    nc.sync.dma_start(
                        exch_in[h, 0:NUMSZ].rearrange("(p f) -> p f", p=C),
                        NumT[:, HALF * h : HALF * h + HALF],
                    )
                    nc.sync.dma_start(
                        exch_in[h, NUMSZ:EXSZ].rearrange("(p f) -> p f", p=1),
                        DenT[:, HALF * h : HALF * h + HALF],
                    )
                if not skip_rs:
                    nc.gpsimd.collective_compute(
                        "ReduceScatter",
                        mybir.AluOpType.add,
                        replica_groups=[[0, 1], [2, 3], [4, 5], [6, 7]],
                        ins=[exch_in.opt()],
                        outs=[exch_out.opt()],
                    )

                    # ---- epilogue: normalize + transpose to [pos, c] rows ----
                    d32 = epi.tile([32, 128], f32, tag="d32", name=f"d32{su}")
                    nc.sync.dma_start(
                        d32[:], exch_out[0, NUMSZ:EXSZ].rearrange("(a b) -> a b", a=32)
                    )
                    dT = evp.tile([128, 32], f32, tag="ev", name=f"dT{su}")
                    nc.tensor.transpose(dT[:], d32[:], ident[0:32, 0:32])
                    dT_sb = epi.tile([128, 32], f32, tag="dTsb", name=f"dTsb{su}")
                    nc.scalar.copy(dT_sb[:], dT[:])
                    recipD = epi.tile([128, 32], f32, tag="recipD", name=f"rD{su}")
                    nc.vector.reciprocal(recipD[:], dT_sb[:])

                    oview = out_half.rearrange("(r m p) c -> p r m c", p=128, m=4)
                    nview = exch_out[0, 0:NUMSZ].rearrange("(p r f) -> p r f", p=C, r=8)
                    for r in range(8):
                        nst = epi.tile([128, 512], f32, tag="nst", name=f"nst{r}{su}")
                        nc.sync.dma_start(nst[:], nview[:, r, :])
                        tp = evp.tile([128, 512], f32, tag="ev", name=f"tp{r}{su}")
                        for mm in range(4):
                            nc.tensor.matmul(
                                tp[:, 128 * mm : 128 * mm + 128],
                                nst[:, 128 * mm : 128 * mm + 128],
                                ident[:],
                                start=True, stop=True,
                                is_transpose=True, skip_group_check=True,
                            )
                        ot = mskp.tile([128, 4, 128], f32, tag="ot", name=f"ot{r}{su}")
                        nc.vector.tensor_tensor(
                            ot[:],
                            tp[:].rearrange("p (m i) -> p m i", m=4),
                            recipD[:, 4 * r : 4 * r + 4, None].to_broadcast([128, 4, 128]),
                            op=mybir.AluOpType.mult,
                        )
                        nc.sync.dma_start(oview[:, r, :, :], ot[:])

            if unroll_k:
                for _u in range(unroll_k):
                    _one_iter(f"_u{_u}")
            elif loop_k:
                with tc.For_i(0, loop_k, 1):
                    _one_iter("")
            else:
                _one_iter("")

    nc.finalize()
    return nc


# ---------------- host side ----------------

_SEG_POS = None


def _seg_positions():
    global _SEG_POS
    if _SEG_POS is None:
        segs = []
        for w, r in zip([2048, 4096, 8192], [1, 2, 4]):
            off = 1 % r
            for start in range(0, N, w):
                segs.append(np.arange(start, start + w)[off::r])
        _SEG_POS = segs  # 7 arrays of 2048
    return _SEG_POS


def _make_masks():
    q = np.arange(128)[:, None]
    k = np.arange(128)[None, :]
    tri = np.where(k <= q, 0.0, NEG).astype(np.float16)
    zero = np.zeros((128, 128), np.float16)
    full = np.full((128, 128), NEG, np.float16)
    m_even = np.concatenate([tri, full], axis=1)   # delta=0: diag chunk first
    m_odd = np.concatenate([zero, tri], axis=1)    # delta=1: diag chunk last
    return m_even, m_odd


_NC = None


def _get_nc():
    global _NC
    if _NC is None:
        _NC = build_nc()
    return _NC


def kernel(x, Wq, Wk, Wv, indices):
    x = np.asarray(x, dtype=np.float32)
    Wq = np.asarray(Wq, dtype=np.float32)
    Wk = np.asarray(Wk, dtype=np.float32)
    Wv = np.asarray(Wv, dtype=np.float32)

    M = (Wq.astype(np.float64) @ Wk.T.astype(np.float64) / math.sqrt(C)).astype(
        np.float16
    )
    Wv16 = Wv.astype(np.float16)
    m_even, m_odd = _make_masks()
    segs = _seg_positions()

    in_maps = []
    for core in range(8):
        b = core // 2
        odd_core = core % 2
        xTb = np.ascontiguousarray(x[b].T.astype(np.float16))  # (C, N)
        bx7 = np.empty((NPIECE, C, SW), np.float16)
        beta7 = np.empty((1, NPIECE), np.int32)
        for p in range(NPIECE):
            # delta: core even -> segs0-3 even-qtiles, segs4-6 odd; odd core flips
            delta = (0 if p < 4 else 1) ^ odd_core
            pos = segs[p]
            bx7[p, :, :S] = xTb[:, pos]
            bx7[p, :, S:] = m_even if delta == 0 else m_odd
            beta7[0, p] = 128 * delta
        in_maps.append(
            {
                "bx7": bx7,
                "beta7": beta7,
                "Mt": M,
                "Wvt": Wv16,
            }
        )

    nc = _get_nc()
    res = run_bass_kernel_spmd(nc, in_maps, list(range(8))).results

    out = np.empty((B, N, C), np.float32)
    for b in range(B):
        out[b, : N // 2] = res[2 * b]["out_half"]
        out[b, N // 2 :] = res[2 * b + 1]["out_half"]
    return out


def kernel_profiled(x, Wq, Wk, Wv, indices, **trace_kwargs):
    """Like kernel() but returns (out, BassKernelResults) with trace enabled."""
    global run_bass_kernel_spmd
    orig = run_bass_kernel_spmd
    holder = {}

    def wrapper(nc, in_maps, core_ids, **kw):
        r = orig(nc, in_maps, core_ids, trace=True, **trace_kwargs)
        holder["r"] = r
        return r

    run_bass_kernel_spmd = wrapper
    try:
        out = kernel(x, Wq, Wk, Wv, indices)
    finally:
        run_bass_kernel_spmd = orig
    return out, holder["r"]


# revision 45
# speedup vs baseline: 1.0372x; 1.0372x over previous
"""Dilated self-attention TRN2 kernel (nn_DilatedSelfAttention).

Problem (hardcoded — self-contained):
  x (4, 8192, 128) f32; Wq/Wk/Wv (128,128) f32; indices (14336) i64.
  WS=[2048,4096,8192], RS=[1,2,4], HEAD_IDX=1 -> 7 segments of 2048 per batch:
    seg0..3: windows [2048t, 2048(t+1))           (stride 1)
    seg4:    1 + 2*i, i<2048   (odd of [0,4096))  (stride 2)
    seg5:    4097 + 2*i        (odd of [4096,8192))
    seg6:    1 + 4*i           (p%4==1)           (stride 4)
  Each segment: causal softmax attention (per-segment row max subtracted),
  outputs mixed position-wise weighted by softmax denominators:
    out[p] = sum_seg (expS @ v)[p] / sum_seg denom[p]   (per-seg max shifts
    folded into both numerator and denominator — matches reference exactly).

Sharding: core pair (2b, 2b+1) owns batch b. Each segment is split into two
half-pieces by query 128-tile parity (delta=0: even qtiles, delta=1: odd).
Every core runs SEVEN structurally identical pieces (uniform SPMD program);
the only per-core data differences are the gathered inputs, the diag masks,
and a dynamic column offset (128*delta) for the output scatter.

All PE inputs are f16 (prepared on host: x^T per segment packed with the
additive -3e4 diag mask into one [C, 2304] DMA payload, M = WqWk^T/sqrt(C),
Wv; -3e4 is equivalent to -1e9 since exp underflows to 0 either way).
Per piece (segment context S=2048, local queries QL=1024 in 8 slots of 128):
  q' = x_seg @ M   [f16 matmul, PSUM->f16 qpt via DVE+ACT]
  v  = x_seg @ Wv  [f16, PSUM->f16 vsl via DVE+ACT]
  slot j: S-row = q'_j @ x over 256(j+1) keys, computed as 512-col blocks
    into SINGLE-BANK [128,512] PSUM tiles (6-buffer ring; one bank each so
    ~6 blocks stay in flight — the release->matmul->rowmax->exp PSUM ring is
    the throughput limiter otherwise), additive diag/pad mask via
    ident16@mask matmul into the last block,
    per-block rowmax partial (DVE) -> negmx -> per-block exp with bias=-mx
    and fused denominator accum (ACT) -> E f16,
    one blocked DMA-xbar transpose E -> ET[k-chunk, q] per slot,
    EV: out^T[c, 128 q] accumulated over the 2(j+1) causal k-chunks (the
    extra chunk vs delta=0's true need holds exp(masked)=0, keeping the
    program uniform across cores).
  EV matmuls are drained with an 8-slot lag AND carry a nosync ordering edge
  onto the most recent slot's S-matmul: the Tile scheduler's internal cost
  model under-estimates DMA-transpose completion (HWDGE 625ns serial issue +
  transfer + ~900ns sem), and without the edge it interleaves EVs early where
  they head-of-line-block the in-order PE queue for ~3us per piece.
  Scatter-add: EV PSUM -> SBUF (DVE), then Pool adds into batch-position
  accumulators at dynamic strided offsets (beta = 128*delta); denominators
  go through a PE transpose + DMA row-flatten (GPSIMD cannot access PSUM).
Pair ReduceScatter sums the two cores' accumulators; each core normalizes and
writes half the batch rows.

Measured (loop-slope on HW, 8 cores): ~191us/iter compute vs ~306us baseline;
TimelineSim (cost-model) 162us with ACT the busiest engine at 74%.
"""
import math
import os
import sys

sys.path.insert(0, "/opt/trn_rl_repo")

import numpy as np

import concourse.bass as bass
import concourse.bacc as bacc
import concourse.mybir as mybir
import concourse.tile as tile
from concourse.bass_utils import run_bass_kernel_spmd
from concourse.masks import make_identity

f32 = mybir.dt.float32
f32r = mybir.dt.float32r
f16 = mybir.dt.float16
i32 = mybir.dt.int32

B, N, C = 4, 8192, 128
S = 2048          # segment length
NCH = 16          # 128-chunks per segment
NSLOT = 8         # q-slots per piece
QL = NSLOT * 128  # 1024 local queries per piece
NPIECE = 7
NEG = -30000.0    # additive mask; exp(s-m+NEG) underflows to 0 exactly

# per piece-slot-index: segment id == piece id; (base, stride) of position map
SEG_BASE = [0, 2048, 4096, 6144, 1, 4097, 1]
SEG_STRIDE = [1, 1, 1, 1, 2, 2, 4]

EV_LAG = 8        # slots between a slot's softmax and its EV matmuls
SW = S + 256      # bx payload cols: x_seg^T (2048) ++ mask (256)


def build_nc(loop_k=None, skip_rs=False, skip=(), unroll_k=None):
    nc = bacc.Bacc(None, target_bir_lowering=False)

    bx7 = nc.dram_tensor("bx7", [NPIECE, C, SW], f16, kind="ExternalInput")
    beta7 = nc.dram_tensor("beta7", [1, NPIECE], i32, kind="ExternalInput")
    Mt = nc.dram_tensor("Mt", [C, C], f16, kind="ExternalInput")
    Wvt = nc.dram_tensor("Wvt", [C, C], f16, kind="ExternalInput")
    out_half = nc.dram_tensor("out_half", [N // 2, C], f32, kind="ExternalOutput")

    HALF = N // 2                      # 4096 positions per core after RS
    NUMSZ = C * HALF                   # 524288
    EXSZ = NUMSZ + HALF                # + DenT half

    with tile.TileContext(nc) as tc:
        with (
            tc.tile_pool(name="fix", bufs=1) as fix,
            tc.tile_pool(name="bx", bufs=2) as bxp,
            tc.tile_pool(name="dr1", bufs=3) as dr1p,
            tc.tile_pool(name="qpr", bufs=2) as qprp,
            tc.tile_pool(name="vsl", bufs=2) as vslp,
            tc.tile_pool(name="msk", bufs=2) as mskp,
            tc.tile_pool(name="ET", bufs=2) as ETp,
            tc.tile_pool(name="E", bufs=6) as Ep,
            tc.tile_pool(name="small", bufs=6) as smp,
            tc.tile_pool(name="dsb", bufs=2) as dsbp,
            tc.tile_pool(name="spool", bufs=7, space="PSUM") as spool,
            tc.tile_pool(name="evp", bufs=1, space="PSUM") as evp,
            tc.tile_pool(name="dram", bufs=1, space="DRAM") as drp,
            tc.tile_pool(name="epi", bufs=1) as epi,
        ):
            # ---- fixed tensors ----
            ident = fix.tile([128, 128], f32)
            make_identity(nc, ident[:])
            ident16 = fix.tile([128, 128], f16)
            nc.gpsimd.tensor_copy(ident16[:], ident[:])

            m16 = fix.tile([C, C], f16)
            wv16 = fix.tile([C, C], f16)
            nc.sync.dma_start(m16[:], Mt[:])
            nc.sync.dma_start(wv16[:], Wvt[:])

            beta_sb = fix.tile([1, NPIECE], i32)
            nc.sync.dma_start(beta_sb[:], beta7[:])

            NumT = fix.tile([C, N], f32)
            DenT = fix.tile([1, N], f32)
            nc.gpsimd.memset(NumT[:], 0.0)
            nc.gpsimd.memset(DenT[:], 0.0)

            exch_in = drp.tile([2, EXSZ], f32)
            exch_out = drp.tile([1, EXSZ], f32)

            def _one_iter(su):
                # ---- per-piece pipeline state ----
                st_bx = [None] * NPIECE
                st_msk = [None] * NPIECE
                st_beta = [None] * NPIECE
                st_qpt = [None] * NPIECE
                st_vsl = [None] * NPIECE
                st_dsl = [None] * NPIECE
                st_ET = [None] * NPIECE
                st_ev = [None] * NPIECE    # (ev_lo, ev_hi) PSUM tiles
                st_numv = [None] * NPIECE
                st_denv = [None] * NPIECE
                st_qxs = [None] * NPIECE
                st_nms = [None] * NPIECE
                st_wrep = [None] * NPIECE
                st_drow = [None] * NPIECE
                anchor = [None]   # last S-matmul inst of most recent slot

                def emit_load(p):
                    bxm = bxp.tile([C, SW], f16, name=f"bx{p}{su}", tag="bx")
                    nc.sync.dma_start(bxm[:], bx7[p])
                    st_bx[p], st_msk[p] = bxm, bxm[:, S:SW]

                    regs = nc.alloc_registers(
                        f"beta_{p}{su}",
                        engines=[mybir.EngineType.Pool, mybir.EngineType.DVE],
                    )
                    nc.regs_load(regs, beta_sb[0:1, p : p + 1])
                    beta = nc.snap(regs, donate=True, min_val=0, max_val=128)
                    st_beta[p] = beta

                    # gather local queries (strided by qtile parity) -> qx;
                    # emitted at load time so it queues on Pool ahead of the
                    # previous pieces' scatter-adds (avoids head-of-line wait)
                    qsrc = (
                        bxm[:, bass.ds(beta, 1920)]
                        .rearrange("p (j i) -> p j i", i=128)[:, 0::2, :]
                    )
                    qx = qprp.tile([C, NSLOT, 128], f16, name=f"qx{p}{su}", tag="qx")
                    nc.gpsimd.tensor_copy(qx[:], qsrc)
                    st_qxs[p] = qx

                def emit_prep(p):
                    bx = st_bx[p]
                    qx = st_qxs[p]
                    beta = st_beta[p]

                    # q' = M @ qx  (PSUM f32 -> f16 qpt via ACT)
                    qpt = qprp.tile([C, QL], f16, tag="qpt", name=f"qpt{p}{su}")
                    for h in range(2):
                        qp_ps = spool.tile([128, 512], f32, tag="s", name=f"qps{p}{h}{su}")
                        nc.tensor.matmul(
                            qp_ps[:],
                            m16[:],
                            qx[:].rearrange("p j i -> p (j i)")[
                                :, 512 * h : 512 * h + 512
                            ],
                            start=True, stop=True, skip_group_check=True,
                        )
                        if h == 0:
                            nc.vector.tensor_copy(qpt[:, 0:512], qp_ps[:])
                        else:
                            nc.scalar.copy(qpt[:, 512:1024], qp_ps[:])

                    # v = x_seg @ Wv  (PSUM f32 -> f16 vsl via Pool)
                    vsl = vslp.tile([128, NCH * 128], f16, name=f"vsl{p}{su}", tag="vsl")
                    for qtr in range(4):
                        v_ps = spool.tile([128, 512], f32, tag="s", name=f"vps{p}{qtr}{su}")
                        for q in range(4):
                            cch = 4 * qtr + q
                            nc.tensor.matmul(
                                v_ps[:, 128 * q : 128 * q + 128],
                                bx[:, 128 * cch : 128 * cch + 128],
                                wv16[:],
                                start=True, stop=True, skip_group_check=True,
                            )
                        if qtr < 3:
                            nc.vector.tensor_copy(
                                vsl[:, 512 * qtr : 512 * qtr + 512], v_ps[:])
                        else:
                            nc.scalar.copy(
                                vsl[:, 512 * qtr : 512 * qtr + 512], v_ps[:])

                    ET = ETp.tile([128, NCH, QL], f16, name=f"ET{p}{su}", tag="ET")
                    denslab = smp.tile([128, NSLOT], f32, tag="denslab", name=f"dsl{p}{su}")
                    st_qpt[p], st_vsl[p], st_dsl[p], st_ET[p] = qpt, vsl, denslab, ET

                    beta = st_beta[p]
                    sstr, sbase = SEG_STRIDE[p], SEG_BASE[p]
                    st_numv[p] = (
                        NumT[:, sbase :: sstr][:, bass.ds(beta, 1920)]
                        .rearrange("p (j i) -> p j i", i=128)[:, 0::2, :]
                    )
                    st_denv[p] = (
                        DenT[:, sbase :: sstr][:, bass.ds(beta, 1920)]
                        .rearrange("p (j i) -> p j i", i=128)[:, 0::2, :]
                    )

                def emit_slot(p, j):
                    bx, mk = st_bx[p], st_msk[p]
                    qpt, denslab, ET = st_qpt[p], st_dsl[p], st_ET[p]
                    ext = 256 * (j + 1)
                    nb = (ext + 511) // 512
                    maxp = smp.tile([128, 4], f32, tag="maxp", name=f"mx{p}{j}{su}")
                    sblocks = []
                    # one single-bank PSUM tile per 512-col block: six blocks
                    # in flight keep the release->matmul->max->exp ring off
                    # the critical path
                    for b in range(nb):
                        w = min(512, ext - 512 * b)
                        stt = spool.tile([128, 512], f32, tag="s", name=f"st{p}_{j}_{b}{su}")
                        sblocks.append((stt, w))
                        mi = nc.tensor.matmul(
                            stt[:, 0:w],
                            qpt[:, 128 * j : 128 * j + 128],
                            bx[:, 512 * b : 512 * b + w],
                            start=True, stop=not (b == nb - 1),
                            skip_group_check=True,
                        )
                        if b == nb - 1:
                            mi = nc.tensor.matmul(
                                stt[:, w - 256 : w],
                                ident16[:],
                                mk,
                                start=False, stop=True, skip_group_check=True,
                            )
                            anchor[0] = mi.ins
                        nc.vector.tensor_reduce(
                            maxp[:, b : b + 1], stt[:, 0:w],
                            axis=mybir.AxisListType.X, op=mybir.AluOpType.max,
                        )
                    negmx = smp.tile([128, 1], f32, tag="negmx", name=f"nm{p}{j}{su}")
                    nc.vector.tensor_reduce(
                        negmx[:], maxp[:, 0:nb],
                        axis=mybir.AxisListType.X, op=mybir.AluOpType.max,
                        negate=True,
                    )

                    Et = Ep.tile([128, S], f16, tag="Et", name=f"Et{p}{j}{su}")
                    denp = smp.tile([128, 4], f32, tag="denp", name=f"dp{p}{j}{su}")
                    for b, (stt, w) in enumerate(sblocks):
                        nc.scalar.activation(
                            Et[:, 512 * b : 512 * b + w],
                            stt[:, 0:w],
                            mybir.ActivationFunctionType.Exp,
                            bias=negmx[:, 0:1], scale=1.0,
                            accum_out=denp[:, b : b + 1],
                        )
                    nc.vector.tensor_reduce(
                        denslab[:, j : j + 1], denp[:, 0:nb],
                        axis=mybir.AxisListType.X, op=mybir.AluOpType.add,
                    )

                    nc.sync.dma_start_transpose(
                        ET[:, 0 : 2 * (j + 1), 128 * j : 128 * j + 128],
                        Et[:, 0:ext],
                    )

                def emit_ev(p, j):
                    vsl, ET = st_vsl[p], st_ET[p]
                    if j % 4 == 0:
                        ev = evp.tile([128, 512], f32, tag="ev",
                                      name=f"ev{p}_{j // 4}{su}")
                        if j < 4:
                            st_ev[p] = [ev, None]
                        else:
                            st_ev[p][1] = ev
                    ev = st_ev[p][j // 4]
                    col = 128 * (j % 4)
                    nch_j = 2 * (j + 1)
                    for cch in range(nch_j):
                        mi = nc.tensor.matmul(
                            ev[:, col : col + 128],
                            vsl[:, 128 * cch : 128 * cch + 128],
                            ET[:, cch, 128 * j : 128 * j + 128],
                            start=(cch == 0),
                            stop=(cch == nch_j - 1),
                            skip_group_check=True,
                        )
                        if cch == 0 and anchor[0] is not None:
                            import bass_rust as _br
                            _br.add_dep_helper(
                                mi.ins, anchor[0], sync=False,
                                reason="keep EV behind the S-matmul stream",
                            )

                def emit_scatter(p, g):
                    # EV PSUM group g (slots 4g..4g+3) -> SBUF, then Pool adds
                    # into the NumT view (GPSIMD cannot touch PSUM)
                    ev = st_ev[p][g]
                    evs = dr1p.tile([C, 512], f32, tag="evs", name=f"evs{p}{g}{su}")
                    nc.vector.tensor_copy(evs[:], ev[:])
                    numv = st_numv[p][:, 4 * g : 4 * g + 4, :]
                    nc.gpsimd.tensor_tensor(
                        numv, numv,
                        evs[:].rearrange("p (j i) -> p j i", i=128),
                        op=mybir.AluOpType.add,
                    )

                def emit_wden(p):
                    denslab = st_dsl[p]
                    dslT = evp.tile([NSLOT, 128], f32, tag="ev", name=f"dslT{p}{su}")
                    nc.tensor.transpose(dslT[:], denslab[:, 0:NSLOT], ident[:])
                    dsl_sb = dsbp.tile([NSLOT, 128], f32, tag="dslsb", name=f"dsb{p}{su}")
                    nc.vector.tensor_copy(dsl_sb[:], dslT[:])
                    denrow = dr1p.tile([1, QL], f32, tag="denrow", name=f"drow{p}{su}")
                    nc.sync.dma_start(denrow[:], dsl_sb[:])
                    st_drow[p] = denrow

                def emit_denadd(p):
                    denv = st_denv[p]
                    nc.gpsimd.tensor_tensor(
                        denv, denv,
                        st_drow[p][:].rearrange("p (j i) -> p j i", i=128),
                        op=mybir.AluOpType.add,
                    )

                # ---- main software-pipelined emission ----
                SLOT_ORDER = list(range(NSLOT))
                pend = []
                ev_done = {}

                def drain_one():
                    pp, jj = pend.pop(0)
                    emit_ev(pp, jj)
                    g = jj // 4
                    k = (pp, g)
                    ev_done[k] = ev_done.get(k, 0) + 1
                    if ev_done[k] == 4:
                        emit_scatter(pp, g)
                        if ev_done.get((pp, 0), 0) == 4 and ev_done.get((pp, 1), 0) == 4:
                            emit_denadd(pp)

                emit_load(0)
                emit_prep(0)
                for p in range(NPIECE):
                    if p + 1 < NPIECE:
                        emit_load(p + 1)
                    for j in SLOT_ORDER:
                        emit_slot(p, j)
                        pend.append((p, j))
                        while len(pend) > EV_LAG:
                            drain_one()
                    emit_wden(p)
                    if p + 1 < NPIECE:
                        emit_prep(p + 1)
                while pend:
                    drain_one()

                # ---- exchange: ReduceScatter over the pair ----
                for h in range(2 if not skip_rs else 0):
                    nc.sync.dma_start(
                        exch_in[h, 0:NUMSZ].rearrange("(p f) -> p f", p=C),
                        NumT[:, HALF * h : HALF * h + HALF],
                    )
                    nc.sync.dma_start(
                        exch_in[h, NUMSZ:EXSZ].rearrange("(p f) -> p f", p=1),
                        DenT[:, HALF * h : HALF * h + HALF],
                    )
                if not skip_rs:
                    nc.gpsimd.collective_compute(
                        "ReduceScatter",
                        mybir.AluOpType.add,
                        replica_groups=[[0, 1], [2, 3], [4, 5], [6, 7]],
                        ins=[exch_in.opt()],
                        outs=[exch_out.opt()],
                    )

                    # ---- epilogue: normalize + transpose to [pos, c] rows ----
                    d32 = epi.tile([32, 128], f32, tag="d32", name=f"d32{su}")
                    nc.sync.dma_start(
                        d32[:], exch_out[0, NUMSZ:EXSZ].rearrange("(a b) -> a b", a=32)
                    )
                    dT = evp.tile([128, 32], f32, tag="ev", name=f"dT{su}")
                    nc.tensor.transpose(dT[:], d32[:], ident[0:32, 0:32])
                    dT_sb = epi.tile([128, 32], f32, tag="dTsb", name=f"dTsb{su}")
                    nc.scalar.copy(dT_sb[:], dT[:])
                    recipD = epi.tile([128, 32], f32, tag="recipD", name=f"rD{su}")
                    nc.vector.reciprocal(recipD[:], dT_sb[:])

                    oview = out_half.rearrange("(r m p) c -> p r m c", p=128, m=4)
                    nview = exch_out[0, 0:NUMSZ].rearrange("(p r f) -> p r f", p=C, r=8)
                    for r in range(8):
                        nst = epi.tile([128, 512], f32, tag="nst", name=f"nst{r}{su}")
                        nc.sync.dma_start(nst[:], nview[:, r, :])
                        tp = evp.tile([128, 512], f32, tag="ev", name=f"tp{r}{su}")
                        for mm in range(4):
                            nc.tensor.matmul(
                                tp[:, 128 * mm : 128 * mm + 128],
                                nst[:, 128 * mm : 128 * mm + 128],
                                ident[:],
                                start=True, stop=True,
                                is_transpose=True, skip_group_check=True,
                            )
                        ot = mskp.tile([128, 4, 128], f32, tag="ot", name=f"ot{r}{su}")
                        nc.vector.tensor_tensor(
                            ot[:],
                            tp[:].rearrange("p (m i) -> p m i", m=4),
                            recipD[:, 4 * r : 4 * r + 4, None].to_broadcast([128, 4, 128]),
                            op=mybir.AluOpType.mult,
                        )
                        nc.sync.dma_start(oview[:, r, :, :], ot[:])

            if unroll_k:
                for _u in range(unroll_k):
                    _one_iter(f"_u{_u}")
            elif loop_k:
                with tc.For_i(0, loop_k, 1):
                    _one_iter("")
            else:
                _one_iter("")

    nc.finalize()
    return nc


# ---------------- host side ----------------

_SEG_POS = None


def _seg_positions():
    global _SEG_POS
    if _SEG_POS is None:
        segs = []
        for w, r in zip([2048, 4096, 8192], [1, 2, 4]):
            off = 1 % r
            for start in range(0, N, w):
                segs.append(np.arange(start, start + w)[off::r])
        _SEG_POS = segs  # 7 arrays of 2048
    return _SEG_POS


def _make_masks():
    q = np.arange(128)[:, None]
    k = np.arange(128)[None, :]
    tri = np.where(k <= q, 0.0, NEG).astype(np.float16)
    zero = np.zeros((128, 128), np.float16)
    full = np.full((128, 128), NEG, np.float16)
    m_even = np.concatenate([tri, full], axis=1)   # delta=0: diag chunk first
    m_odd = np.concatenate([zero, tri], axis=1)    # delta=1: diag chunk last
    return m_even, m_odd


_NC = None


def _get_nc():
    global _NC
    if _NC is None:
        _NC = build_nc()
    return _NC


def kernel(x, Wq, Wk, Wv, indices):
    x = np.asarray(x, dtype=np.float32)
    Wq = np.asarray(Wq, dtype=np.float32)
    Wk = np.asarray(Wk, dtype=np.float32)
    Wv = np.asarray(Wv, dtype=np.float32)

    M = (Wq.astype(np.float64) @ Wk.T.astype(np.float64) / math.sqrt(C)).astype(
        np.float16
    )
    Wv16 = Wv.astype(np.float16)
    m_even, m_odd = _make_masks()
    segs = _seg_positions()

    in_maps = []
    for core in range(8):
        b = core // 2
        odd_core = core % 2
        xTb = np.ascontiguousarray(x[b].T.astype(np.float16))  # (C, N)
        bx7 = np.empty((NPIECE, C, SW), np.float16)
        beta7 = np.empty((1, NPIECE), np.int32)
        for p in range(NPIECE):
            # delta: core even -> segs0-3 even-qtiles, segs4-6 odd; odd core flips
            delta = (0 if p < 4 else 1) ^ odd_core
            pos = segs[p]
            bx7[p, :, :S] = xTb[:, pos]
            bx7[p, :, S:] = m_even if delta == 0 else m_odd
            beta7[0, p] = 128 * delta
        in_maps.append(
            {
                "bx7": bx7,
                "beta7": beta7,
                "Mt": M,
                "Wvt": Wv16,
            }
        )

    nc = _get_nc()
    res = run_bass_kernel_spmd(nc, in_maps, list(range(8))).results

    out = np.empty((B, N, C), np.float32)
    for b in range(B):
        out[b, : N // 2] = res[2 * b]["out_half"]
        out[b, N // 2 :] = res[2 * b + 1]["out_half"]
    return out


def kernel_profiled(x, Wq, Wk, Wv, indices, **trace_kwargs):
    """Like kernel() but returns (out, BassKernelResults) with trace enabled."""
    global run_bass_kernel_spmd
    orig = run_bass_kernel_spmd
    holder = {}

    def wrapper(nc, in_maps, core_ids, **kw):
        r = orig(nc, in_maps, core_ids, trace=True, **trace_kwargs)
        holder["r"] = r
        return r

    run_bass_kernel_spmd = wrapper
    try:
        out = kernel(x, Wq, Wk, Wv, indices)
    finally:
        run_bass_kernel_spmd = orig
    return out, holder["r"]
